# revision 16
# baseline (speedup 1.0000x reference)
"""Causal multi-head self-attention with RoPE on 8 Trainium2 NeuronCores.

Problem: b=4, s=2048, d_model=1024, 16 heads, dk=64, causal, RoPE(theta=1e4).

Sharding: 8 cores = (batch, head-half). Core c handles batch c//2 and heads
(c%2)*8 .. +8: QKV projections, causal attention, partial output projection;
the host sums the two partials per batch.

Layouts (host-prepared): x^T, W^T per-core slices (x/W cast to bf16 on host,
including Wo); Q/K head-dims pre-permuted to [evens; odds] blocks so RoPE is
Q*cos + swap32(Q)*sin (32-partition DMA swaps). Scores are computed in [k, q]
layout; softmax denominators ride a [V|1] column in the PV matmul; the 1/sum
normalization is folded in before the output projection.

v3 scheduling: the attention inner loop is exp-bound on the Scalar engine, so
the projections for groups 1-3 and V tiles 8-15 are deferred and fed into the
tensor-engine stream between attention steps (a few matmuls per step, with
deadline drains). That keeps the PE stream dense end-to-end, which both hides
the projection cost inside exp-waits and keeps the HAM clock gate at 8/8.
RoPE chains run on vector+gpsimd a group ahead; the softmax-denominator bounce
(outT -> reciprocal -> lhs) runs per chunk so nothing serializes at group
boundaries.

Precision: all matmuls bf16 with fp32 PSUM accumulate. Overall rel err vs the
fp32 reference ~1e-2 (bf16 Q/K rounding dominates).
"""
import sys
import numpy as np

for _p in ('/root/.axon_site/_ro/trn_rl_repo', '/opt/trn_rl_repo'):
    if _p not in sys.path:
        sys.path.append(_p)

import concourse.bass as bass
import concourse.tile as tile
from concourse import bacc, mybir
from concourse.bass_utils import run_bass_kernel_spmd

F32 = mybir.dt.float32
F32R = mybir.dt.float32r
BF16 = mybir.dt.bfloat16
EXP = mybir.ActivationFunctionType.Exp

B, S, D = 4, 2048, 1024
NH, DK = 16, 64
NHC = 8            # heads per core
HD = NHC * DK      # 512
NG = 4             # head-pairs per core
NC_CHUNK = 512     # q-chunk
N_CHUNKS = S // NC_CHUNK
N_KT = S // 128
KSUB = D // 128
THETA = 10000.0

_CACHED = {}


class _Feeder:
    """Deferred projection work, fed into the PE stream between attention steps.

    Groups are (tag, [ops...]) where each op is a closure emitting one
    instruction. A group's 8 matmuls accumulate into one PSUM tile; the 9th op
    evacuates it. emit(n) emits up to n ops; drain(tag) emits through the last
    group carrying that tag.
    """

    def __init__(self):
        self.groups = []     # list of (tag, [op, ...])
        self.gi = 0          # current group index
        self.oi = 0          # next op within current group

    def push(self, tag, ops):
        self.groups.append((tag, ops))

    def _step(self):
        if self.gi >= len(self.groups):
            return False
        tag, ops = self.groups[self.gi]
        ops[self.oi]()
        self.oi += 1
        if self.oi == len(ops):
            self.gi += 1
            self.oi = 0
        return True

    def emit(self, n, allow=None):
        for _ in range(n):
            if self.gi >= len(self.groups):
                return
            if (allow is not None and self.oi == 0
                    and not allow(self.groups[self.gi][0])):
                return
            self._step()

    def drain(self, tag):
        last = max((i for i, (t, _) in enumerate(self.groups) if t == tag),
                   default=-1)
        while self.gi <= last:
            self._step()

    def drain_all(self):
        while self._step():
            pass


def _build():
    nc = bacc.Bacc('TRN2', target_bir_lowering=False, debug=False, num_devices=8)
    xT = nc.dram_tensor('xT', [D, S], BF16, kind='ExternalInput').ap()
    wqT = nc.dram_tensor('wqT', [D, HD], BF16, kind='ExternalInput').ap()
    wkT = nc.dram_tensor('wkT', [D, HD], BF16, kind='ExternalInput').ap()
    wvT = nc.dram_tensor('wvT', [D, HD], BF16, kind='ExternalInput').ap()
    woT = nc.dram_tensor('woT', [HD, D], BF16, kind='ExternalInput').ap()
    cosd = nc.dram_tensor('cosd', [128, S], F32, kind='ExternalInput').ap()
    sind = nc.dram_tensor('sind', [128, S], F32, kind='ExternalInput').ap()
    maskd = nc.dram_tensor('maskd', [128, 128], BF16, kind='ExternalInput').ap()
    y = nc.dram_tensor('y', [S, D], F32, kind='ExternalOutput').ap()
    recip_d = nc.dram_tensor('recip_d', [NHC, S], F32).ap()     # 1/sums

    with tile.TileContext(nc) as tc:
        with tc.tile_pool(name='persist', bufs=1) as persist:
            q_sb = persist.tile([128, NG, S], BF16, tag='q_sb')
            k_sb = persist.tile([128, NG, S], BF16, tag='k_sb')
            v_sb = persist.tile([128, N_KT, NHC * (DK + 1)], BF16, tag='v_sb')
            v4 = v_sb.rearrange('p t (h x) -> p t h x', h=NHC)
            lhs_sb = persist.tile([128, NG, S], BF16, tag='lhs_sb')
            wo_sb = persist.tile([128, NG, D], BF16, tag='wo_sb')
            cos_sb = persist.tile([128, S], F32, tag='cos_sb')
            sin_sb = persist.tile([128, S], F32, tag='sin_sb')
            tri_sb = persist.tile([128, 128], BF16, tag='tri_sb')

            with tc.tile_pool(name='xw', bufs=1) as xw, \
                 tc.tile_pool(name='ropetmp', bufs=1) as ropetmp:
                x_sb = xw.tile([128, KSUB, S], BF16, tag='x_sb')
                for s in range(KSUB):
                    nc.sync.dma_start(x_sb[:, s], xT[128 * s:128 * (s + 1), :])
                wq_sb = xw.tile([128, KSUB, HD], BF16, tag='wq_sb')
                wk_sb = xw.tile([128, KSUB, HD], BF16, tag='wk_sb')
                wv_sb = xw.tile([128, KSUB, HD], BF16, tag='wv_sb')
                for s in range(KSUB):
                    sl = slice(128 * s, 128 * (s + 1))
                    nc.scalar.dma_start(wq_sb[:, s], wqT[sl, :])
                    nc.gpsimd.dma_start(wk_sb[:, s], wkT[sl, :])
                    nc.scalar.dma_start(wv_sb[:, s], wvT[sl, :])
                # tables after the weights (gpsimd queue order matters)
                nc.gpsimd.dma_start(cos_sb[:], cosd)
                nc.gpsimd.dma_start(sin_sb[:], sind)
                nc.gpsimd.dma_start(tri_sb[:], maskd)
                for g in range(NG):
                    nc.gpsimd.dma_start(wo_sb[:, g], woT[128 * g:128 * (g + 1), :])

                def rope(g):
                    for t_sb in (q_sb, k_sb):
                        ch = t_sb[:, g, :]
                        sw = ropetmp.tile([128, S], BF16, tag='swap')
                        for blk in range(4):
                            src = (blk // 2) * 64 + (1 - blk % 2) * 32
                            nc.sync.dma_start(sw[32 * blk:32 * (blk + 1), :],
                                              ch[src:src + 32, :])
                        t1 = ropetmp.tile([128, S], F32, tag='t1')
                        nc.vector.tensor_mul(t1[:], ch, cos_sb[:])
                        t2 = ropetmp.tile([128, S], F32, tag='t2')
                        nc.gpsimd.tensor_tensor(t2[:], sw[:], sin_sb[:],
                                                mybir.AluOpType.mult)
                        nc.vector.tensor_add(ch, t1[:], t2[:])

                # ---- phase 1: Q/K for group 0, V tiles 0-7, RoPE(0) ----
                with tc.tile_pool(name='ps1', bufs=4, space='PSUM') as ps1:
                    for w_sb, dst in ((wq_sb, q_sb), (wk_sb, k_sb)):
                        for c in range(N_CHUNKS):
                            ps = ps1.tile([128, NC_CHUNK], F32, tag='ps1')
                            for s in range(KSUB):
                                nc.tensor.matmul(
                                    ps[:], w_sb[:, s, 0:128],
                                    x_sb[:, s, NC_CHUNK * c:NC_CHUNK * (c + 1)],
                                    start=(s == 0), stop=(s == KSUB - 1))
                            dstc = dst[:, 0, NC_CHUNK * c:NC_CHUNK * (c + 1)]
                            if c % 2 == 0:
                                nc.vector.tensor_copy(dstc, ps[:])
                            else:
                                nc.scalar.copy(dstc, ps[:])
                    rope(0)
                    ones_sb = persist.tile([128, N_KT, NHC], BF16, tag='ones')
                    nc.vector.memset(ones_sb[:], 1.0)
                    nc.vector.tensor_copy(v4[:, :, :, DK], ones_sb[:])
                    for t in range(8):
                        ps = ps1.tile([128, NC_CHUNK], F32, tag='ps1')
                        for s in range(KSUB):
                            nc.tensor.matmul(
                                ps[:], x_sb[:, s, 128 * t:128 * (t + 1)],
                                wv_sb[:, s], start=(s == 0), stop=(s == KSUB - 1))
                        if t % 2 == 0:
                            nc.vector.tensor_copy(
                                v4[:, t, :, 0:DK],
                                ps.rearrange('p (h m) -> p h m', h=NHC))
                        else:
                            nc.scalar.copy(
                                v4[:, t, :, 0:DK],
                                ps.rearrange('p (h m) -> p h m', h=NHC))

                # ---- phase 2: attention with deferred projections fed in ----
                with tc.tile_pool(name='pp', bufs=4) as pp, \
                     tc.tile_pool(name='stage', bufs=3) as stage, \
                     tc.tile_pool(name='rcpool', bufs=3) as rcpool, \
                     tc.tile_pool(name='scps', bufs=2, space='PSUM') as scps, \
                     tc.tile_pool(name='outps', bufs=3, space='PSUM') as outps, \
                     tc.tile_pool(name='projps', bufs=1, space='PSUM') as projps:

                    feeder = _Feeder()

                    def v_group(t):
                        box = {}
                        def mm(s):
                            def op():
                                if s == 0:
                                    box['ps'] = projps.tile(
                                        [128, NC_CHUNK], F32, tag='pps',
                                        name='pps')
                                nc.tensor.matmul(
                                    box['ps'][:], x_sb[:, s, 128 * t:128 * (t + 1)],
                                    wv_sb[:, s], start=(s == 0),
                                    stop=(s == KSUB - 1))
                            return op
                        def evac():
                            nc.vector.tensor_copy(
                                v4[:, t, :, 0:DK],
                                box['ps'].rearrange('p (h m) -> p h m', h=NHC))
                        return [mm(s) for s in range(KSUB)] + [evac]

                    def qk_group(w_sb, g, c, dst):
                        box = {}
                        def mm(s):
                            def op():
                                if s == 0:
                                    box['ps'] = projps.tile(
                                        [128, NC_CHUNK], F32, tag='pps',
                                        name='pps')
                                nc.tensor.matmul(
                                    box['ps'][:], w_sb[:, s, 128 * g:128 * (g + 1)],
                                    x_sb[:, s, NC_CHUNK * c:NC_CHUNK * (c + 1)],
                                    start=(s == 0), stop=(s == KSUB - 1))
                            return op
                        def evac():
                            nc.vector.tensor_copy(
                                dst[:, g, NC_CHUNK * c:NC_CHUNK * (c + 1)],
                                box['ps'][:])
                        return [mm(s) for s in range(KSUB)] + [evac]

                    def oproj_group(t, half):
                        box = {}
                        ts_ = slice(128 * t, 128 * (t + 1))
                        hs = slice(512 * half, 512 * (half + 1))
                        def mm(g):
                            def op():
                                if g == 0:
                                    box['ps'] = projps.tile(
                                        [128, 512], F32, tag='pps', name='pps')
                                nc.tensor.matmul(
                                    box['ps'][:], lhs_sb[:, g, ts_],
                                    wo_sb[:, g, hs],
                                    start=(g == 0), stop=(g == NG - 1))
                            return op
                        def evac():
                            yt = stage.tile([128, 512], F32, tag='yt',
                                            name='yt')
                            nc.vector.tensor_copy(yt[:], box['ps'][:])
                            nc.sync.dma_start(y[ts_, hs], yt[:])
                        return [mm(g) for g in range(NG)] + [evac]

                    for t in range(8, N_KT):
                        feeder.push(('v', t), v_group(t))
                    for g in range(1, NG):
                        for c in range(N_CHUNKS):
                            feeder.push(('q', g), qk_group(wq_sb, g, c, q_sb))
                        for c in range(N_CHUNKS):
                            feeder.push(('k', g), qk_group(wk_sb, g, c, k_sb))
                    for t in range(12):
                        for half in range(2):
                            feeder.push(('yo', t), oproj_group(t, half))
                    PACE = {0: 4, 1: 2, 2: 2, 3: 5}
                    for g in range(NG):
                        hA, hB = 2 * g, 2 * g + 1
                        for c in range(N_CHUNKS):
                            cs = slice(NC_CHUNK * c, NC_CHUNK * (c + 1))
                            if g == 0 and c >= 2:       # V deadline for this chunk
                                feeder.drain(('v', 4 * c + 3))
                            oA = outps.tile([DK + 1, NC_CHUNK], F32, tag='o',
                                            name='oA')
                            oB = outps.tile([DK + 1, NC_CHUNK], F32, tag='o',
                                            name='oB')
                            n_kt = 4 * (c + 1)
                            for kt in range(n_kt):
                                if g < 3 and c == 3 and kt == 4:
                                    # next group's Q/K must land; RoPE it now
                                    feeder.drain(('q', g + 1))
                                    feeder.drain(('k', g + 1))
                                    rope(g + 1)
                                ks = slice(128 * kt, 128 * (kt + 1))
                                j = kt - 4 * c          # >=0 on the diagonal
                                v0 = max(j, 0) * 128    # first valid q column
                                sA = slice(v0, NC_CHUNK)
                                sB = slice(NC_CHUNK + v0, 2 * NC_CHUNK)
                                st_, sp_ = (kt == 0), (kt == n_kt - 1)
                                qv = q_sb[:, g,
                                          NC_CHUNK * c + v0:NC_CHUNK * (c + 1)]
                                sc = scps.tile([128, 2 * NC_CHUNK], F32, tag='sc')
                                nc.tensor.matmul(sc[:, sA], k_sb[0:64, g, ks],
                                                 qv[0:64], start=True, stop=True)
                                nc.tensor.matmul(sc[:, sB], k_sb[64:128, g, ks],
                                                 qv[64:128], start=True, stop=True)
                                feeder.emit(
                                    0 if (g == 3 and c == 0) else PACE[g],
                                    allow=lambda tag, g=g, c=c:
                                        tag[0] != 'yo'
                                        or (g == 3 and tag[1] < 4 * c))
                                p = pp.tile([128, 2 * NC_CHUNK], BF16, tag='p')
                                if v0 <= 128:
                                    # one call; cols [0,v0) are unread garbage
                                    nc.scalar.activation(
                                        out=p[:], in_=sc[:], func=EXP,
                                        scale=1.0 / np.sqrt(DK))
                                else:
                                    nc.scalar.activation(
                                        out=p[:, sA], in_=sc[:, sA], func=EXP,
                                        scale=1.0 / np.sqrt(DK))
                                    nc.scalar.activation(
                                        out=p[:, sB], in_=sc[:, sB], func=EXP,
                                        scale=1.0 / np.sqrt(DK))
                                if j >= 0:       # triangle on the diag sub-block
                                    dseg = bass.AP(
                                        tensor=p.tensor, offset=p.offset + v0,
                                        ap=[list(p.ap[0]), [NC_CHUNK, 2], [1, 128]])
                                    nc.vector.tensor_tensor(
                                        dseg, dseg,
                                        tri_sb[:, None, :].to_broadcast([128, 2, 128]),
                                        mybir.AluOpType.mult)
                                nc.tensor.matmul(oA[:, sA],
                                                 v_sb[:, kt, 65 * hA:65 * (hA + 1)],
                                                 p[:, sA], start=st_, stop=sp_)
                                nc.tensor.matmul(oB[:, sA],
                                                 v_sb[:, kt, 65 * hB:65 * (hB + 1)],
                                                 p[:, sB], start=st_, stop=sp_)
                            st2 = stage.tile([DK + 1, 2 * NC_CHUNK], F32R, tag='st2')
                            nc.vector.tensor_copy(st2[:, 0:NC_CHUNK], oA[:])
                            nc.vector.tensor_copy(st2[:, NC_CHUNK:], oB[:])

                            # per-chunk normalization, all on-chip: stack the
                            # two heads' V rows; broadcast 1/sums across
                            # partitions with a TensorE outer product
                            with nc.allow_low_precision(
                                    reason='f32r tiles hold f32 bits here'):
                                nc.vector.reciprocal(st2[64:65, :],
                                                     st2[64:65, :])
                            ot = rcpool.tile([128, NC_CHUNK], F32R, tag='ot')
                            nc.sync.dma_start(ot[0:64, :], st2[0:DK, 0:NC_CHUNK])
                            nc.sync.dma_start(ot[64:128, :], st2[0:DK, NC_CHUNK:])
                            for i, h in ((0, hA), (1, hB)):
                                nc.sync.dma_start(
                                    bass.AP(tensor=recip_d.tensor,
                                            offset=recip_d.offset + h * S
                                            + NC_CHUNK * c,
                                            ap=[[1, 1], [1, NC_CHUNK]]),
                                    st2[64:65,
                                        NC_CHUNK * i:NC_CHUNK * (i + 1)]
                                    .bitcast(F32))
                            rc = rcpool.tile([128, NC_CHUNK], F32, tag='rc')
                            for half, h in ((0, hA), (1, hB)):
                                nc.sync.dma_start(
                                    rc[64 * half:64 * (half + 1), :],
                                    bass.AP(tensor=recip_d.tensor,
                                            offset=recip_d.offset + h * S
                                            + NC_CHUNK * c,
                                            ap=[[0, 64], [1, NC_CHUNK]]))
                            with nc.allow_low_precision(
                                    reason='f32r tiles hold f32 bits here'):
                                nc.vector.tensor_mul(lhs_sb[:, g, cs], ot[:],
                                                     rc[:])
                    feeder.drain_all()

            # ---------------- phase 3: output projection ----------
            with tc.tile_pool(name='ytmp', bufs=4) as ytmp, \
                 tc.tile_pool(name='ps4', bufs=4, space='PSUM') as ps4:
                for t in range(12, N_KT):
                    ts_ = slice(128 * t, 128 * (t + 1))
                    for half in range(2):
                        ps = ps4.tile([128, 512], F32, tag='ps4')
                        for g in range(NG):
                            nc.tensor.matmul(ps[:], lhs_sb[:, g, ts_],
                                             wo_sb[:, g, 512 * half:512 * (half + 1)],
                                             start=(g == 0), stop=(g == NG - 1))
                        yt = ytmp.tile([128, 512], F32, tag='yt')
                        nc.vector.tensor_copy(yt[:], ps[:])
                        nc.sync.dma_start(y[ts_, 512 * half:512 * (half + 1)], yt[:])
    nc.compile()
    return nc


def _host_inputs(x, Wq, Wk, Wv, Wo, token_positions):
    """Per-core input maps (host-side sharding / layout / dtype prep only)."""
    import ml_dtypes
    bf16 = ml_dtypes.bfloat16
    perm = np.empty(DK, np.int64)
    perm[0:32] = np.arange(0, DK, 2)
    perm[32:64] = np.arange(1, DK, 2)

    inv_freq = 1.0 / (THETA ** (np.arange(0, DK, 2, dtype=np.float64) / DK))  # [32]
    ang = token_positions.astype(np.float64)[None, :] * inv_freq[:, None]     # [32, S]
    cos32 = np.cos(ang).astype(np.float32)
    sin32 = np.sin(ang).astype(np.float32)
    cos128 = np.tile(cos32, (4, 1))
    sin128 = np.concatenate([-sin32, sin32, -sin32, sin32], axis=0)

    tri = (np.arange(128)[None, :] >= np.arange(128)[:, None]).astype(bf16)

    in_maps = []
    for core in range(8):
        b = core // 2
        h0 = (core % 2) * NHC
        cols = slice(h0 * DK, (h0 + NHC) * DK)
        wq_s = Wq[cols, :].reshape(NHC, DK, D)[:, perm, :].reshape(HD, D)
        wk_s = Wk[cols, :].reshape(NHC, DK, D)[:, perm, :].reshape(HD, D)
        in_maps.append({
            'xT': np.ascontiguousarray(x[b].T.astype(bf16)),
            'wqT': np.ascontiguousarray(wq_s.T.astype(bf16)),
            'wkT': np.ascontiguousarray(wk_s.T.astype(bf16)),
            'wvT': np.ascontiguousarray(Wv[cols, :].T.astype(bf16)),
            'woT': np.ascontiguousarray(Wo[:, cols].T.astype(bf16)),
            'cosd': cos128, 'sind': sin128, 'maskd': tri,
        })
    return in_maps


def kernel(x, Wq, Wk, Wv, Wo, token_positions, _results_hook=None):
    if 'nc' not in _CACHED:
        _CACHED['nc'] = _build()
    nc = _CACHED['nc']
    in_maps = _host_inputs(np.asarray(x), np.asarray(Wq), np.asarray(Wk),
                           np.asarray(Wv), np.asarray(Wo),
                           np.asarray(token_positions))
    res = run_bass_kernel_spmd(nc, in_maps, list(range(8)),
                               **(_results_hook or {}))
    if _results_hook is not None:
        _CACHED['last'] = res
    out = np.empty((B, S, D), np.float32)
    for b in range(B):
        out[b] = res.results[2 * b]['y'] + res.results[2 * b + 1]['y']
    return out


# revision 18
# speedup vs baseline: 1.3998x; 1.3998x over previous
"""Causal multi-head self-attention with RoPE on 8 Trainium2 NeuronCores.

Problem: b=4, s=2048, d_model=1024, 16 heads, dk=64, causal, RoPE(theta=1e4).

Sharding: 8 cores = (batch, head-half). Core c handles batch c//2 and heads
(c%2)*8 .. +8: QKV projections, causal attention, partial output projection;
the host sums the two partials per batch.

Layouts (host-prepared): x^T, W^T per-core slices (x/W cast to bf16 on host,
including Wo); Q/K head-dims pre-permuted to [evens; odds] blocks so RoPE is
Q*cos + swap32(Q)*sin (32-partition DMA swaps). Scores are computed in [k, q]
layout; softmax denominators ride a [V|1] column in the PV matmul; the 1/sum
normalization is folded in before the output projection.

v3 scheduling: the attention inner loop is exp-bound on the Scalar engine, so
the projections for groups 1-3 and V tiles 8-15 are deferred and fed into the
tensor-engine stream between attention steps (a few matmuls per step, with
deadline drains). That keeps the PE stream dense end-to-end, which both hides
the projection cost inside exp-waits and keeps the HAM clock gate at 8/8.
RoPE chains run on vector+gpsimd a group ahead; the softmax-denominator bounce
(outT -> reciprocal -> lhs) runs per chunk so nothing serializes at group
boundaries.

Precision: all matmuls bf16 with fp32 PSUM accumulate. Overall rel err vs the
fp32 reference ~1e-2 (bf16 Q/K rounding dominates).
"""
import sys
import numpy as np

for _p in ('/root/.axon_site/_ro/trn_rl_repo', '/opt/trn_rl_repo'):
    if _p not in sys.path:
        sys.path.append(_p)

import concourse.bass as bass
import concourse.tile as tile
from concourse import bacc, mybir
from concourse.bass_utils import run_bass_kernel_spmd

F32 = mybir.dt.float32
F32R = mybir.dt.float32r
BF16 = mybir.dt.bfloat16
EXP = mybir.ActivationFunctionType.Exp

B, S, D = 4, 2048, 1024
NH, DK = 16, 64
NHC = 8            # heads per core
HD = NHC * DK      # 512
NG = 4             # head-pairs per core
NC_CHUNK = 512     # q-chunk
N_CHUNKS = S // NC_CHUNK
N_KT = S // 128
KSUB = D // 128
THETA = 10000.0

_CACHED = {}


class _Feeder:
    """Deferred projection work, fed into the PE stream between attention steps.

    Groups are (tag, [ops...]) where each op is a closure emitting one
    instruction. A group's 8 matmuls accumulate into one PSUM tile; the 9th op
    evacuates it. emit(n) emits up to n ops; drain(tag) emits through the last
    group carrying that tag.
    """

    def __init__(self):
        self.groups = []     # list of (tag, [op, ...])
        self.gi = 0          # current group index
        self.oi = 0          # next op within current group

    def push(self, tag, ops):
        self.groups.append((tag, ops))

    def _step(self):
        if self.gi >= len(self.groups):
            return False
        tag, ops = self.groups[self.gi]
        ops[self.oi]()
        self.oi += 1
        if self.oi == len(ops):
            self.gi += 1
            self.oi = 0
        return True

    def emit(self, n, allow=None):
        for _ in range(n):
            if self.gi >= len(self.groups):
                return
            if (allow is not None and self.oi == 0
                    and not allow(self.groups[self.gi][0])):
                return
            self._step()

    def drain(self, tag):
        last = max((i for i, (t, _) in enumerate(self.groups) if t == tag),
                   default=-1)
        while self.gi <= last:
            self._step()

    def drain_all(self):
        while self._step():
            pass


def _build():
    nc = bacc.Bacc('TRN2', target_bir_lowering=False, debug=False, num_devices=8)
    xT = nc.dram_tensor('xT', [D, S], BF16, kind='ExternalInput').ap()
    wqT = nc.dram_tensor('wqT', [D, HD], BF16, kind='ExternalInput').ap()
    wkT = nc.dram_tensor('wkT', [D, HD], BF16, kind='ExternalInput').ap()
    wvT = nc.dram_tensor('wvT', [D, HD], BF16, kind='ExternalInput').ap()
    woT = nc.dram_tensor('woT', [HD, D], BF16, kind='ExternalInput').ap()
    cosd = nc.dram_tensor('cosd', [128, S], F32, kind='ExternalInput').ap()
    sind = nc.dram_tensor('sind', [128, S], F32, kind='ExternalInput').ap()
    maskd = nc.dram_tensor('maskd', [128, 128], BF16, kind='ExternalInput').ap()
    y = nc.dram_tensor('y', [S, D], F32, kind='ExternalOutput').ap()
    sums_d = nc.dram_tensor('sums_d', [NHC, S], F32).ap()      # raw sums
    recip_d = nc.dram_tensor('recip_d', [NHC, S], F32).ap()     # 1/sums

    with tile.TileContext(nc) as tc:
        with tc.tile_pool(name='persist', bufs=1) as persist:
            q_sb = persist.tile([128, NG, S], BF16, tag='q_sb')
            k_sb = persist.tile([128, NG, S], BF16, tag='k_sb')
            v_sb = persist.tile([128, N_KT, NHC * (DK + 1)], BF16, tag='v_sb')
            v4 = v_sb.rearrange('p t (h x) -> p t h x', h=NHC)
            lhs_sb = persist.tile([128, NG, S], BF16, tag='lhs_sb')
            wo_sb = persist.tile([128, NG, D], BF16, tag='wo_sb')
            cos_sb = persist.tile([128, S], F32, tag='cos_sb')
            sin_sb = persist.tile([128, S], F32, tag='sin_sb')
            tri_sb = persist.tile([128, 128], BF16, tag='tri_sb')

            with tc.tile_pool(name='xw', bufs=1) as xw, \
                 tc.tile_pool(name='ropetmp', bufs=1) as ropetmp:
                x_sb = xw.tile([128, KSUB, S], BF16, tag='x_sb')
                for s in range(KSUB):
                    nc.sync.dma_start(x_sb[:, s], xT[128 * s:128 * (s + 1), :])
                wq_sb = xw.tile([128, KSUB, HD], BF16, tag='wq_sb')
                wk_sb = xw.tile([128, KSUB, HD], BF16, tag='wk_sb')
                wv_sb = xw.tile([128, KSUB, HD], BF16, tag='wv_sb')
                for s in range(KSUB):
                    sl = slice(128 * s, 128 * (s + 1))
                    nc.scalar.dma_start(wq_sb[:, s], wqT[sl, :])
                    nc.gpsimd.dma_start(wk_sb[:, s], wkT[sl, :])
                    nc.scalar.dma_start(wv_sb[:, s], wvT[sl, :])
                # tables after the weights (gpsimd queue order matters)
                nc.gpsimd.dma_start(cos_sb[:], cosd)
                nc.gpsimd.dma_start(sin_sb[:], sind)
                nc.gpsimd.dma_start(tri_sb[:], maskd)
                for g in range(NG):
                    nc.gpsimd.dma_start(wo_sb[:, g], woT[128 * g:128 * (g + 1), :])

                def rope(g):
                    for t_sb in (q_sb, k_sb):
                        ch = t_sb[:, g, :]
                        sw = ropetmp.tile([128, S], BF16, tag='swap')
                        for blk in range(4):
                            src = (blk // 2) * 64 + (1 - blk % 2) * 32
                            nc.sync.dma_start(sw[32 * blk:32 * (blk + 1), :],
                                              ch[src:src + 32, :])
                        t1 = ropetmp.tile([128, S], F32, tag='t1')
                        nc.vector.tensor_mul(t1[:], ch, cos_sb[:])
                        t2 = ropetmp.tile([128, S], F32, tag='t2')
                        nc.gpsimd.tensor_tensor(t2[:], sw[:], sin_sb[:],
                                                mybir.AluOpType.mult)
                        nc.vector.tensor_add(ch, t1[:], t2[:])

                # ---- phase 1: Q/K for group 0, V tiles 0-7, RoPE(0) ----
                with tc.tile_pool(name='ps1', bufs=4, space='PSUM') as ps1:
                    for w_sb, dst in ((wq_sb, q_sb), (wk_sb, k_sb)):
                        for c in range(N_CHUNKS):
                            ps = ps1.tile([128, NC_CHUNK], F32, tag='ps1')
                            for s in range(KSUB):
                                nc.tensor.matmul(
                                    ps[:], w_sb[:, s, 0:128],
                                    x_sb[:, s, NC_CHUNK * c:NC_CHUNK * (c + 1)],
                                    start=(s == 0), stop=(s == KSUB - 1))
                            dstc = dst[:, 0, NC_CHUNK * c:NC_CHUNK * (c + 1)]
                            if c % 2 == 0:
                                nc.vector.tensor_copy(dstc, ps[:])
                            else:
                                nc.scalar.copy(dstc, ps[:])
                    rope(0)
                    ones_sb = persist.tile([128, N_KT, NHC], BF16, tag='ones')
                    nc.vector.memset(ones_sb[:], 1.0)
                    nc.vector.tensor_copy(v4[:, :, :, DK], ones_sb[:])
                    for t in range(8):
                        ps = ps1.tile([128, NC_CHUNK], F32, tag='ps1')
                        for s in range(KSUB):
                            nc.tensor.matmul(
                                ps[:], x_sb[:, s, 128 * t:128 * (t + 1)],
                                wv_sb[:, s], start=(s == 0), stop=(s == KSUB - 1))
                        if t % 2 == 0:
                            nc.vector.tensor_copy(
                                v4[:, t, :, 0:DK],
                                ps.rearrange('p (h m) -> p h m', h=NHC))
                        else:
                            nc.scalar.copy(
                                v4[:, t, :, 0:DK],
                                ps.rearrange('p (h m) -> p h m', h=NHC))

                # ---- phase 2: attention with deferred projections fed in ----
                with tc.tile_pool(name='pp', bufs=4) as pp, \
                     tc.tile_pool(name='stage', bufs=3) as stage, \
                     tc.tile_pool(name='rcpool', bufs=3) as rcpool, \
                     tc.tile_pool(name='scps', bufs=2, space='PSUM') as scps, \
                     tc.tile_pool(name='outps', bufs=3, space='PSUM') as outps, \
                     tc.tile_pool(name='projps', bufs=1, space='PSUM') as projps:

                    feeder = _Feeder()

                    def v_group(t):
                        box = {}
                        def mm(s):
                            def op():
                                if s == 0:
                                    box['ps'] = projps.tile(
                                        [128, NC_CHUNK], F32, tag='pps',
                                        name='pps')
                                nc.tensor.matmul(
                                    box['ps'][:], x_sb[:, s, 128 * t:128 * (t + 1)],
                                    wv_sb[:, s], start=(s == 0),
                                    stop=(s == KSUB - 1))
                            return op
                        def evac():
                            nc.vector.tensor_copy(
                                v4[:, t, :, 0:DK],
                                box['ps'].rearrange('p (h m) -> p h m', h=NHC))
                        return [mm(s) for s in range(KSUB)] + [evac]

                    def qk_group(w_sb, g, c, dst):
                        box = {}
                        def mm(s):
                            def op():
                                if s == 0:
                                    box['ps'] = projps.tile(
                                        [128, NC_CHUNK], F32, tag='pps',
                                        name='pps')
                                nc.tensor.matmul(
                                    box['ps'][:], w_sb[:, s, 128 * g:128 * (g + 1)],
                                    x_sb[:, s, NC_CHUNK * c:NC_CHUNK * (c + 1)],
                                    start=(s == 0), stop=(s == KSUB - 1))
                            return op
                        def evac():
                            nc.vector.tensor_copy(
                                dst[:, g, NC_CHUNK * c:NC_CHUNK * (c + 1)],
                                box['ps'][:])
                        return [mm(s) for s in range(KSUB)] + [evac]

                    def oproj_group(t, half):
                        box = {}
                        ts_ = slice(128 * t, 128 * (t + 1))
                        hs = slice(512 * half, 512 * (half + 1))
                        def mm(g):
                            def op():
                                if g == 0:
                                    box['ps'] = projps.tile(
                                        [128, 512], F32, tag='pps', name='pps')
                                nc.tensor.matmul(
                                    box['ps'][:], lhs_sb[:, g, ts_],
                                    wo_sb[:, g, hs],
                                    start=(g == 0), stop=(g == NG - 1))
                            return op
                        def evac():
                            yt = stage.tile([128, 512], F32, tag='yt',
                                            name='yt')
                            nc.vector.tensor_copy(yt[:], box['ps'][:])
                            nc.sync.dma_start(y[ts_, hs], yt[:])
                        return [mm(g) for g in range(NG)] + [evac]

                    for t in range(8, N_KT):
                        feeder.push(('v', t), v_group(t))
                    for g in range(1, NG):
                        for c in range(N_CHUNKS):
                            feeder.push(('q', g), qk_group(wq_sb, g, c, q_sb))
                        for c in range(N_CHUNKS):
                            feeder.push(('k', g), qk_group(wk_sb, g, c, k_sb))
                    for t in range(12):
                        for half in range(2):
                            feeder.push(('yo', t), oproj_group(t, half))
                    PACE = {0: 4, 1: 2, 2: 2, 3: 5}
                    for g in range(NG):
                        hA, hB = 2 * g, 2 * g + 1
                        for c in range(N_CHUNKS):
                            cs = slice(NC_CHUNK * c, NC_CHUNK * (c + 1))
                            if g == 0 and c >= 2:       # V deadline for this chunk
                                feeder.drain(('v', 4 * c + 3))
                            oA = outps.tile([DK + 1, NC_CHUNK], F32, tag='o',
                                            name='oA')
                            oB = outps.tile([DK + 1, NC_CHUNK], F32, tag='o',
                                            name='oB')
                            n_kt = 4 * (c + 1)
                            for kt in range(n_kt):
                                if g < 3 and c == 3 and kt == 4:
                                    # next group's Q/K must land; RoPE it now
                                    feeder.drain(('q', g + 1))
                                    feeder.drain(('k', g + 1))
                                    rope(g + 1)
                                ks = slice(128 * kt, 128 * (kt + 1))
                                j = kt - 4 * c          # >=0 on the diagonal
                                v0 = max(j, 0) * 128    # first valid q column
                                sA = slice(v0, NC_CHUNK)
                                sB = slice(NC_CHUNK + v0, 2 * NC_CHUNK)
                                st_, sp_ = (kt == 0), (kt == n_kt - 1)
                                qv = q_sb[:, g,
                                          NC_CHUNK * c + v0:NC_CHUNK * (c + 1)]
                                sc = scps.tile([128, 2 * NC_CHUNK], F32, tag='sc')
                                nc.tensor.matmul(sc[:, sA], k_sb[0:64, g, ks],
                                                 qv[0:64], start=True, stop=True)
                                nc.tensor.matmul(sc[:, sB], k_sb[64:128, g, ks],
                                                 qv[64:128], start=True, stop=True)
                                feeder.emit(
                                    0 if (g == 3 and c == 0) else PACE[g],
                                    allow=lambda tag, g=g, c=c:
                                        tag[0] != 'yo'
                                        or (g == 3 and tag[1] < 4 * c))
                                p = pp.tile([128, 2 * NC_CHUNK], BF16, tag='p')
                                if v0 <= 128:
                                    # one call; cols [0,v0) are unread garbage
                                    nc.scalar.activation(
                                        out=p[:], in_=sc[:], func=EXP,
                                        scale=1.0 / np.sqrt(DK))
                                else:
                                    nc.scalar.activation(
                                        out=p[:, sA], in_=sc[:, sA], func=EXP,
                                        scale=1.0 / np.sqrt(DK))
                                    nc.scalar.activation(
                                        out=p[:, sB], in_=sc[:, sB], func=EXP,
                                        scale=1.0 / np.sqrt(DK))
                                if j >= 0:       # triangle on the diag sub-block
                                    dseg = bass.AP(
                                        tensor=p.tensor, offset=p.offset + v0,
                                        ap=[list(p.ap[0]), [NC_CHUNK, 2], [1, 128]])
                                    nc.vector.tensor_tensor(
                                        dseg, dseg,
                                        tri_sb[:, None, :].to_broadcast([128, 2, 128]),
                                        mybir.AluOpType.mult)
                                nc.tensor.matmul(oA[:, sA],
                                                 v_sb[:, kt, 65 * hA:65 * (hA + 1)],
                                                 p[:, sA], start=st_, stop=sp_)
                                nc.tensor.matmul(oB[:, sA],
                                                 v_sb[:, kt, 65 * hB:65 * (hB + 1)],
                                                 p[:, sB], start=st_, stop=sp_)
                            st2 = stage.tile([DK + 1, 2 * NC_CHUNK], F32R, tag='st2')
                            nc.vector.tensor_copy(st2[:, 0:NC_CHUNK], oA[:])
                            nc.vector.tensor_copy(st2[:, NC_CHUNK:], oB[:])

                            # per-chunk normalization, all on-chip: stack the
                            # two heads' V rows; broadcast 1/sums across
                            # partitions with a TensorE outer product
                            ot = rcpool.tile([128, NC_CHUNK], F32R, tag='ot')
                            nc.gpsimd.dma_start(ot[0:64, :],
                                                st2[0:DK, 0:NC_CHUNK])
                            nc.gpsimd.dma_start(ot[64:128, :],
                                                st2[0:DK, NC_CHUNK:])
                            for i, h in ((0, hA), (1, hB)):
                                nc.sync.dma_start(
                                    bass.AP(tensor=sums_d.tensor,
                                            offset=sums_d.offset + h * S
                                            + NC_CHUNK * c,
                                            ap=[[1, 1], [1, NC_CHUNK]]),
                                    st2[64:65,
                                        NC_CHUNK * i:NC_CHUNK * (i + 1)]
                                    .bitcast(F32))
                            rcp = stage.tile([8, 128], F32, tag='rcp')
                            for i, h in ((0, hA), (1, hB)):
                                nc.sync.dma_start(
                                    rcp[4 * i:4 * (i + 1), :],
                                    bass.AP(tensor=sums_d.tensor,
                                            offset=sums_d.offset + h * S
                                            + NC_CHUNK * c,
                                            ap=[[128, 4], [1, 128]]))
                            nc.vector.reciprocal(rcp[:], rcp[:])
                            for i, h in ((0, hA), (1, hB)):
                                nc.sync.dma_start(
                                    bass.AP(tensor=recip_d.tensor,
                                            offset=recip_d.offset + h * S
                                            + NC_CHUNK * c,
                                            ap=[[128, 4], [1, 128]]),
                                    rcp[4 * i:4 * (i + 1), :])
                            rc = rcpool.tile([128, NC_CHUNK], F32, tag='rc')
                            for half, h in ((0, hA), (1, hB)):
                                nc.sync.dma_start(
                                    rc[64 * half:64 * (half + 1), :],
                                    bass.AP(tensor=recip_d.tensor,
                                            offset=recip_d.offset + h * S
                                            + NC_CHUNK * c,
                                            ap=[[0, 64], [1, NC_CHUNK]]))
                            with nc.allow_low_precision(
                                    reason='f32r tiles hold f32 bits here'):
                                nc.vector.tensor_mul(lhs_sb[:, g, cs], ot[:],
                                                     rc[:])
                    feeder.drain_all()

            # ---------------- phase 3: output projection ----------
            with tc.tile_pool(name='ytmp', bufs=4) as ytmp, \
                 tc.tile_pool(name='ps4', bufs=4, space='PSUM') as ps4:
                for t in range(12, N_KT):
                    ts_ = slice(128 * t, 128 * (t + 1))
                    for half in range(2):
                        ps = ps4.tile([128, 512], F32, tag='ps4')
                        for g in range(NG):
                            nc.tensor.matmul(ps[:], lhs_sb[:, g, ts_],
                                             wo_sb[:, g, 512 * half:512 * (half + 1)],
                                             start=(g == 0), stop=(g == NG - 1))
                        yt = ytmp.tile([128, 512], F32, tag='yt')
                        nc.vector.tensor_copy(yt[:], ps[:])
                        nc.sync.dma_start(y[ts_, 512 * half:512 * (half + 1)], yt[:])
    nc.compile()
    return nc


def _host_inputs(x, Wq, Wk, Wv, Wo, token_positions):
    """Per-core input maps (host-side sharding / layout / dtype prep only)."""
    import ml_dtypes
    bf16 = ml_dtypes.bfloat16
    perm = np.empty(DK, np.int64)
    perm[0:32] = np.arange(0, DK, 2)
    perm[32:64] = np.arange(1, DK, 2)

    inv_freq = 1.0 / (THETA ** (np.arange(0, DK, 2, dtype=np.float64) / DK))  # [32]
    ang = token_positions.astype(np.float64)[None, :] * inv_freq[:, None]     # [32, S]
    cos32 = np.cos(ang).astype(np.float32)
    sin32 = np.sin(ang).astype(np.float32)
    cos128 = np.tile(cos32, (4, 1))
    sin128 = np.concatenate([-sin32, sin32, -sin32, sin32], axis=0)

    tri = (np.arange(128)[None, :] >= np.arange(128)[:, None]).astype(bf16)

    in_maps = []
    for core in range(8):
        b = core // 2
        h0 = (core % 2) * NHC
        cols = slice(h0 * DK, (h0 + NHC) * DK)
        wq_s = Wq[cols, :].reshape(NHC, DK, D)[:, perm, :].reshape(HD, D)
        wk_s = Wk[cols, :].reshape(NHC, DK, D)[:, perm, :].reshape(HD, D)
        in_maps.append({
            'xT': np.ascontiguousarray(x[b].T.astype(bf16)),
            'wqT': np.ascontiguousarray(wq_s.T.astype(bf16)),
            'wkT': np.ascontiguousarray(wk_s.T.astype(bf16)),
            'wvT': np.ascontiguousarray(Wv[cols, :].T.astype(bf16)),
            'woT': np.ascontiguousarray(Wo[:, cols].T.astype(bf16)),
            'cosd': cos128, 'sind': sin128, 'maskd': tri,
        })
    return in_maps


def kernel(x, Wq, Wk, Wv, Wo, token_positions, _results_hook=None):
    if 'nc' not in _CACHED:
        _CACHED['nc'] = _build()
    nc = _CACHED['nc']
    in_maps = _host_inputs(np.asarray(x), np.asarray(Wq), np.asarray(Wk),
                           np.asarray(Wv), np.asarray(Wo),
                           np.asarray(token_positions))
    res = run_bass_kernel_spmd(nc, in_maps, list(range(8)),
                               **(_results_hook or {}))
    if _results_hook is not None:
        _CACHED['last'] = res
    out = np.empty((B, S, D), np.float32)
    for b in range(B):
        out[b] = res.results[2 * b]['y'] + res.results[2 * b + 1]['y']
    return out
